# revision 32
# baseline (speedup 1.0000x reference)
"""SO(3)-equivariant conv block on 8 trn2 NeuronCores.

Strategy:
  - Host (numpy): l=1 tensor-product features, concat -> xin [B,144,20,20,20],
    zero-pad to 26^3, build dense kernel K [72,144,7,7,7] from CG basis and
    per-call weights, reshape into matmul-friendly layouts.
  - Shard: 8 cores = (batch 2) x (z-chunks of 5 output planes). Each core gets
    an 11-plane padded input slab (halo 3 each side) and the full K.
  - Device: per output z-plane, accumulate 392 fp32r matmuls into PSUM
    (343 taps x [K=128 chans] + 49 (dz,dy) taps x [K=112 = 16 chans x 7 dx,
    pre-shifted "im2col" replica built on host]). M=72 C_out, N=400=(20y,20x).
    Then norm-based BN: per-core partial sum-of-squares -> AllReduce [72] ->
    group-sum via tiny matmul with 0/1 matrix -> rsqrt -> scale; bias+ReLU on
    the 8 scalar channels. Output [72, 5*400] per core, host reassembles.
"""

import os
import numpy as np
from math import factorial, pi, sqrt

SIZE, PAD, N_RADIAL = 7, 3, 3
M = 8
B_, D_ = 2, 20
C_IN = 72          # M*1 + M*3 + M*5
C_MID = 144        # 2*M*(1+3+5)
C_OUT = 72
DP = D_ + 2 * PAD  # 26
ZSH = 5            # output z-planes per core
ZSLAB = ZSH + SIZE - 1  # 11 input planes per core
N_CORES = 8

# ---------- Clebsch-Gordan / spherical harmonics (build-time, numpy) ----------

def _su2_cg(j1, j2, j3, m1, m2, m3):
    if m1 + m2 != m3 or not abs(j1 - j2) <= j3 <= j1 + j2:
        return 0.0
    f = factorial
    pref = sqrt((2*j3+1) * f(j3+j1-j2) * f(j3-j1+j2) * f(j1+j2-j3) / f(j1+j2+j3+1))
    pref *= sqrt(f(j3+m3)*f(j3-m3)*f(j1-m1)*f(j1+m1)*f(j2-m2)*f(j2+m2))
    s = 0.0
    for k in range(0, j1+j2+j3+1):
        d = [k, j1+j2-j3-k, j1-m1-k, j2+m2-k, j3-j2+m1+k, j3-j1-m2+k]
        if min(d) >= 0:
            s += (-1)**k / np.prod([float(f(t)) for t in d])
    return pref * s

def _u_c2r(l):
    U = np.zeros((2*l+1, 2*l+1), dtype=complex)
    U[l, l] = 1.0
    for m in range(1, l+1):
        U[l+m, l+m] = (-1)**m / sqrt(2); U[l+m, l-m] = 1.0/sqrt(2)
        U[l-m, l-m] = 1j/sqrt(2);        U[l-m, l+m] = -1j*(-1)**m/sqrt(2)
    return U

def _real_cg(l1, l2, l3):
    Cc = np.zeros((2*l1+1, 2*l2+1, 2*l3+1))
    for m1 in range(-l1, l1+1):
        for m2 in range(-l2, l2+1):
            for m3 in range(-l3, l3+1):
                Cc[m1+l1, m2+l2, m3+l3] = _su2_cg(l1, l2, l3, m1, m2, m3)
    T = np.einsum('ac,im,jn,mnc->aij', _u_c2r(l3), _u_c2r(l1).conj(), _u_c2r(l2).conj(), Cc)
    return (T.imag if np.abs(T.imag).max() > np.abs(T.real).max() else T.real)

def _real_sph_all(lmax, x, y, z):
    r = np.sqrt(x*x + y*y + z*z); eps = 1e-9
    ct = np.where(r > eps, z / np.maximum(r, eps), 1.0)
    phi = np.arctan2(y, x)
    st = np.sqrt(np.clip(1.0 - ct*ct, 0.0, None))
    P = {(0, 0): np.ones_like(ct)}
    for m in range(1, lmax+1):
        dfac = float(np.prod(np.arange(1, 2*m, 2)))
        P[(m, m)] = (-1)**m * dfac * st**m
    for m in range(0, lmax):
        P[(m+1, m)] = ct * (2*m+1) * P[(m, m)]
    for m in range(0, lmax+1):
        for l in range(m+2, lmax+1):
            P[(l, m)] = ((2*l-1)*ct*P[(l-1, m)] - (l+m-1)*P[(l-2, m)]) / (l - m)
    Y = {}
    for l in range(lmax+1):
        arr = np.zeros((2*l+1,) + ct.shape)
        for m in range(0, l+1):
            N = sqrt((2*l+1)/(4*pi) * factorial(l-m)/factorial(l+m))
            if m == 0:
                arr[l] = N * P[(l, 0)]
            else:
                arr[l+m] = sqrt(2)*N*P[(l, m)]*np.cos(m*phi)
                arr[l-m] = sqrt(2)*N*P[(l, m)]*np.sin(m*phi)
        if l > 0:
            arr = np.where(r[None] > eps, arr, 0.0)
        Y[l] = arr
    return Y

_g = np.arange(SIZE, dtype=np.float64) - PAD
_GX, _GY, _GZ = np.meshgrid(_g, _g, _g, indexing='ij')
_R = np.sqrt(_GX**2 + _GY**2 + _GZ**2)
_Y = _real_sph_all(4, _GX, _GY, _GZ)
_RAD = np.stack([np.exp(-0.5*(_R - c)**2) for c in np.linspace(0.0, PAD, N_RADIAL)]) * (_R <= PAD + 0.5)

def _build_basis(lo, li):
    ls = list(range(abs(lo-li), lo+li+1))
    out = []
    for l in ls:
        T = _real_cg(li, l, lo)
        ang = np.einsum('aim,mxyz->aixyz', T, _Y[l])
        for j in range(N_RADIAL):
            Bk = ang * _RAD[j]
            n = np.linalg.norm(Bk)
            out.append(Bk/n if n > 1e-12 else Bk)
    return np.stack(out).reshape(len(ls), N_RADIAL, 2*lo+1, 2*li+1, SIZE, SIZE, SIZE).astype(np.float32)

BASIS = {(lo, li): _build_basis(lo, li) for lo in range(3) for li in range(3)}
TPC = {l: _real_cg(1, 1, l).astype(np.float32) for l in range(3)}

# BN group matrix: 72 channels -> 24 multiplicity groups
_G = np.zeros((C_OUT, 3*M), dtype=np.float32)
for m_ in range(M):
    _G[m_, m_] = 1.0
    _G[M + 3*m_: M + 3*m_ + 3, M + m_] = 1.0
    _G[4*M + 5*m_: 4*M + 5*m_ + 5, 2*M + m_] = 1.0

# ---------------------------- host-side compute ----------------------------

def _build_K(ws):
    """ws: dict[(lo,li)] -> w [M, 2M, nl, 3]. Returns K [72, 144, 7,7,7]."""
    rows = []
    for lo in range(3):
        blocks = []
        for li in range(3):
            w = ws[(lo, li)]
            B = BASIS[(lo, li)]  # [nl, 3, 2lo+1, 2li+1, 7,7,7]
            Kb = np.einsum('uvlj,ljaixyz->uavixyz', w, B, optimize=True)
            blocks.append(Kb.reshape(M*(2*lo+1), 2*M*(2*li+1), SIZE, SIZE, SIZE))
        rows.append(np.concatenate(blocks, axis=1))
    return np.concatenate(rows, axis=0)

def _host_features(x):
    """x [2,72,20,20,20] -> padded xin [2,144,26,26,26] (float32)."""
    b = x.shape[0]
    s = x[:, :M]
    v = x[:, M:4*M].reshape(b, M, 3, D_, D_, D_)
    q = x[:, 4*M:].reshape(b, M, 5, D_, D_, D_)
    irr = [np.einsum('aij,buixyz,bujxyz->buaxyz', TPC[l], v, v, optimize=True)
           for l in range(3)]
    in0 = np.concatenate([s[:, :, None], irr[0]], 1).reshape(b, 2*M, D_, D_, D_)
    in1 = np.concatenate([v, irr[1]], 1).reshape(b, 2*M*3, D_, D_, D_)
    in2 = np.concatenate([q, irr[2]], 1).reshape(b, 2*M*5, D_, D_, D_)
    xin = np.concatenate([in0, in1, in2], 1)
    xp = np.zeros((b, C_MID, DP, DP, DP), dtype=np.float32)
    xp[:, :, PAD:PAD+D_, PAD:PAD+D_, PAD:PAD+D_] = xin
    return xp

# ------------------------------ device kernel ------------------------------

_BUILT = None

def _build_nc(reps=1):
    import concourse.bass as bass
    import concourse.bacc as bacc
    import concourse.mybir as mybir
    import concourse.tile as tile
    from contextlib import nullcontext

    f32 = mybir.dt.float32
    bf16 = mybir.dt.bfloat16
    NT2 = SIZE * SIZE                # 49
    NPLANE = 400                     # 20*20 output voxels per plane
    NQ = 8                           # K chunks: (18 chans x 7 dx) = 126 rows
    KC = 126
    CW = ZSLAB * DP * D_             # 5720 per-chunk free size

    nc = bacc.Bacc(None)
    xi_d = nc.declare_dram_parameter("xi", [NQ * KC, CW], bf16, isOutput=False)
    w_d = nc.declare_dram_parameter("w", [NQ * KC, NT2 * C_OUT], bf16, isOutput=False)
    gT_d = nc.declare_dram_parameter("gT", [C_OUT, 3*M], f32, isOutput=False)
    g2_d = nc.declare_dram_parameter("g2", [3*M, C_OUT], f32, isOutput=False)
    gam_d = nc.declare_dram_parameter("gamma", [3*M, 1], f32, isOutput=False)
    cst_d = nc.declare_dram_parameter("cst", [3*M, 2], f32, isOutput=False)
    bias_d = nc.declare_dram_parameter("bias", [M, 1], f32, isOutput=False)
    y_d = nc.declare_dram_parameter("y", [C_OUT, ZSH * NPLANE], f32, isOutput=True)

    with tile.TileContext(nc) as tc:
        with (
            tc.tile_pool(name="wpool", bufs=1) as wpool,
            tc.tile_pool(name="xpool", bufs=1) as xpool,
            tc.tile_pool(name="ypool", bufs=1) as ypool,
            tc.tile_pool(name="spool", bufs=1) as spool,
            tc.tile_pool(name="scratch", bufs=2) as scpool,
            tc.tile_pool(name="psum", bufs=1, space="PSUM") as ppool,
            tc.tile_pool(name="dram", bufs=1, space="DRAM") as dpool,
        ):
            # ---- persistent SBUF tensors ----
            w_sb = [wpool.tile([KC, NT2 * C_OUT], bf16, tag=f"w{q}", name=f"w{q}")
                    for q in range(NQ)]
            x_sb = [xpool.tile([KC, ZSLAB, DP, D_], bf16, tag=f"x{q}", name=f"x{q}")
                    for q in range(NQ)]
            y_sb = ypool.tile([C_OUT, ZSH * NPLANE], f32, tag="y")
            gT_sb = spool.tile([C_OUT, 3*M], f32, tag="gT")
            g2_sb = spool.tile([3*M, C_OUT], f32, tag="g2")
            gam_sb = spool.tile([3*M, 1], f32, tag="gam")
            cst_sb = spool.tile([3*M, 2], f32, tag="cst")
            bias_sb = spool.tile([M, 1], f32, tag="bias")
            ssq5 = spool.tile([C_OUT, ZSH], f32, tag="ssq5")
            ssq_t = spool.tile([C_OUT, 1], f32, tag="ssqt")
            ssq_ar = spool.tile([C_OUT, 1], f32, tag="ssqar")
            sc24 = spool.tile([3*M, 1], f32, tag="sc24")
            sc72 = spool.tile([C_OUT, 1], f32, tag="sc72")

            # ---- input DMAs (per K-chunk so compute starts early) ----
            for q in range(NQ):
                nc.sync.dma_start(w_sb[q][:], w_d[q*KC:(q+1)*KC, :])
                nc.sync.dma_start(x_sb[q][:].rearrange("p a b c -> p (a b c)"),
                                  xi_d[q*KC:(q+1)*KC, :])
            nc.sync.dma_start(gT_sb[:], gT_d[:])
            nc.sync.dma_start(g2_sb[:], g2_d[:])
            nc.sync.dma_start(gam_sb[:], gam_d[:])
            nc.sync.dma_start(cst_sb[:], cst_d[:])
            nc.sync.dma_start(bias_sb[:], bias_d[:])

            rep_ctx = tc.For_i(0, reps, 1) if reps > 1 else nullcontext()
            rep_ctx.__enter__()

            # ---- conv: one PSUM bank per output plane, zo-outer so each bank
            # ---- sees one long uninterrupted accumulation run ----
            n_mm = NQ * NT2  # per-plane matmul count: 8 chunks x 49 (dz,dy)
            for zo in range(ZSH):
                psum = ppool.tile([C_OUT, NPLANE], f32, tag="ps", bufs=3,
                                  name=f"ps{zo}")
                k = 0
                for q in range(NQ):
                    for dz in range(SIZE):
                        for dy in range(SIZE):
                            t = dz*SIZE + dy
                            nc.tensor.matmul(
                                psum[:], w_sb[q][:, t*C_OUT:(t+1)*C_OUT],
                                x_sb[q][:, zo+dz, dy:dy+D_, :],
                                start=(k == 0), stop=(k == n_mm - 1))
                            k += 1
                # evacuate + partial sum-of-squares (overlaps next plane's MMs)
                nc.vector.tensor_copy(y_sb[:, zo*NPLANE:(zo+1)*NPLANE], psum[:])
                sq = scpool.tile([C_OUT, NPLANE], f32, tag="sq")
                nc.scalar.activation(sq[:], psum[:],
                                     mybir.ActivationFunctionType.Square,
                                     accum_out=ssq5[:, zo:zo+1])
            nc.vector.reduce_sum(ssq_t[:], ssq5[:], axis=mybir.AxisListType.X)

            rep_ctx.__exit__(None, None, None)

            # ---- AllReduce the [72,1] partial stats ----
            ar_in = dpool.tile([C_OUT, 1], f32, tag="arin")
            ar_out = dpool.tile([C_OUT, 1], f32, tag="arout")
            nc.gpsimd.dma_start(ar_in[:], ssq_t[:])
            nc.gpsimd.collective_compute(
                "AllReduce", mybir.AluOpType.add,
                replica_groups=[list(range(N_CORES))],
                ins=[ar_in.opt()], outs=[ar_out.opt()])
            nc.gpsimd.dma_start(ssq_ar[:], ar_out[:])

            # ---- BN scale: group-sum, rsqrt, expand, apply ----
            g24 = ppool.tile([3*M, 1], f32, tag="g24")
            nc.tensor.matmul(g24[:], gT_sb[:], ssq_ar[:], start=True, stop=True)
            nc.vector.tensor_scalar(sc24[:], g24[:], cst_sb[:, 0:1], cst_sb[:, 1:2],
                                    op0=mybir.AluOpType.mult,
                                    op1=mybir.AluOpType.add)
            nc.scalar.activation(sc24[:], sc24[:],
                                 mybir.ActivationFunctionType.Sqrt)
            nc.vector.reciprocal(sc24[:], sc24[:])
            nc.vector.tensor_mul(sc24[:], sc24[:], gam_sb[:])
            s72 = ppool.tile([C_OUT, 1], f32, tag="s72")
            nc.tensor.matmul(s72[:], g2_sb[:], sc24[:], start=True, stop=True)
            nc.vector.tensor_copy(sc72[:], s72[:])
            nc.vector.tensor_scalar_mul(y_sb[:], y_sb[:], sc72[:, 0:1])
            nc.scalar.activation(y_sb[:M, :], y_sb[:M, :],
                                 mybir.ActivationFunctionType.Relu,
                                 bias=bias_sb[:, 0:1])

            nc.sync.dma_start(y_d[:], y_sb[:])

    if os.environ.get("KERNEL_LDW_DEDUP", "1") == "1":
        # Consecutive matmuls sharing the stationary operand skip the reload.
        nskip = 0
        for bb in nc.main_func.blocks:
            last = None
            for ins in bb.instructions:
                if type(ins).__name__ == "InstMatmult":
                    w = ins.ins[1]
                    key = (w.memref, w.offset)
                    if key == last:
                        ins.ldweights = False
                        nskip += 1
                    else:
                        last = key
        print(f"[kernel] ldweights dedup: skipped {nskip} reloads")

    nc.finalize()
    return nc

def _get_built(reps=1):
    global _BUILT
    if _BUILT is None:
        _BUILT = {}
    if reps not in _BUILT:
        _BUILT[reps] = _build_nc(reps)
    return _BUILT[reps]

# ------------------------------- entry point -------------------------------

def _prepare_in_maps(x, ws, bn_gamma, bias):
    x = np.asarray(x, dtype=np.float32)
    ws = {k: np.asarray(v, dtype=np.float32) for k, v in ws.items()}
    bn_gamma = np.asarray(bn_gamma, dtype=np.float32)
    bias = np.asarray(bias, dtype=np.float32)

    import ml_dtypes
    bf16 = ml_dtypes.bfloat16

    xp = _host_features(x)                       # [2,144,26,26,26]
    K = _build_K(ws).reshape(C_OUT, C_MID, SIZE, SIZE, SIZE)

    # lhsT layout: rows (c,dx) = c*7+dx, cols ((dz*7+dy)*72 + co)
    w = np.ascontiguousarray(
        K.transpose(1, 4, 2, 3, 0).reshape(C_MID * SIZE, SIZE*SIZE*C_OUT)
    ).astype(bf16)                                                # [1008, 49*72]

    # dx-replicated input (im2col over x): rows (c,dx), contiguous (y,x) blocks
    xr = np.ascontiguousarray(
        np.stack([xp[:, :, :, :, dx:dx+D_] for dx in range(SIZE)], axis=2)
        .reshape(B_, C_MID * SIZE, DP, DP, D_)).astype(bf16)      # [2,1008,26,26,20]

    gamma24 = bn_gamma.reshape(3*M, 1)
    bias8 = bias.reshape(M, 1)

    in_maps = []
    for k in range(N_CORES):
        b, zc = k // 4, k % 4
        z0 = ZSH * zc
        in_maps.append({
            "xi": np.ascontiguousarray(xr[b, :, z0:z0+ZSLAB]).reshape(C_MID*SIZE, -1),
            "w": w,
            "gT": _G, "g2": np.ascontiguousarray(_G.T),
            "gamma": gamma24, "bias": bias8,
            "cst": np.tile(np.array([[1.0/16000.0, 1e-5]], dtype=np.float32),
                           (3*M, 1)),
        })
    return in_maps


_LDW_PATCHED = False

def _maybe_patch_ldw_opt():
    """Optionally enable walrus's ldweights optimization (off by default in
    concourse) — overlaps/dedupes the per-matmul weight loads."""
    global _LDW_PATCHED
    import os
    if _LDW_PATCHED or os.environ.get("KERNEL_LDW_OPT", "0") != "1":
        return
    import concourse.bass_utils as _bu
    orig = _bu.run_command
    def patched(cmd, *a, **kw):
        if isinstance(cmd, list):
            cmd = ["--enable-ldw-opt=true" if c == "--enable-ldw-opt=false" else c
                   for c in cmd]
        return orig(cmd, *a, **kw)
    _bu.run_command = patched
    _LDW_PATCHED = True


def kernel(x, w00, w01, w02, w10, w11, w12, w20, w21, w22, bn_gamma, bias):
    from concourse.bass_utils import run_bass_kernel_spmd
    _maybe_patch_ldw_opt()

    ws = {(0,0): w00, (0,1): w01, (0,2): w02, (1,0): w10, (1,1): w11,
          (1,2): w12, (2,0): w20, (2,1): w21, (2,2): w22}
    in_maps = _prepare_in_maps(x, ws, bn_gamma, bias)
    nc = _get_built()
    res = run_bass_kernel_spmd(nc, in_maps, list(range(N_CORES)))

    out = np.empty((B_, C_OUT, D_, D_, D_), dtype=np.float32)
    for k in range(N_CORES):
        b, zc = k // 4, k % 4
        z0 = ZSH * zc
        out[b, :, z0:z0+ZSH] = res.results[k]["y"].reshape(C_OUT, ZSH, D_, D_)
    return out


# revision 39
# speedup vs baseline: 1.7565x; 1.7565x over previous
"""SO(3)-equivariant conv block on 8 trn2 NeuronCores.

Strategy:
  - Host (numpy): l=1 tensor-product features, concat -> xin [B,144,20,20,20],
    zero-pad to 26^3, build dense kernel K [72,144,7,7,7] from CG basis and
    per-call weights, reshape into matmul-friendly layouts.
  - Shard: 8 cores = (batch 2) x (z-chunks of 5 output planes). Each core gets
    an 11-plane padded input slab (halo 3 each side) and the full K.
  - Device: per output z-plane, accumulate 392 fp32r matmuls into PSUM
    (343 taps x [K=128 chans] + 49 (dz,dy) taps x [K=112 = 16 chans x 7 dx,
    pre-shifted "im2col" replica built on host]). M=72 C_out, N=400=(20y,20x).
    Then norm-based BN: per-core partial sum-of-squares -> AllReduce [72] ->
    group-sum via tiny matmul with 0/1 matrix -> rsqrt -> scale; bias+ReLU on
    the 8 scalar channels. Output [72, 5*400] per core, host reassembles.
"""

import os
import numpy as np
from math import factorial, pi, sqrt

SIZE, PAD, N_RADIAL = 7, 3, 3
M = 8
B_, D_ = 2, 20
C_IN = 72          # M*1 + M*3 + M*5
C_MID = 144        # 2*M*(1+3+5)
C_OUT = 72
DP = D_ + 2 * PAD  # 26
ZSH = 5            # output z-planes per core
ZSLAB = ZSH + SIZE - 1  # 11 input planes per core
N_CORES = 8

# ---------- Clebsch-Gordan / spherical harmonics (build-time, numpy) ----------

def _su2_cg(j1, j2, j3, m1, m2, m3):
    if m1 + m2 != m3 or not abs(j1 - j2) <= j3 <= j1 + j2:
        return 0.0
    f = factorial
    pref = sqrt((2*j3+1) * f(j3+j1-j2) * f(j3-j1+j2) * f(j1+j2-j3) / f(j1+j2+j3+1))
    pref *= sqrt(f(j3+m3)*f(j3-m3)*f(j1-m1)*f(j1+m1)*f(j2-m2)*f(j2+m2))
    s = 0.0
    for k in range(0, j1+j2+j3+1):
        d = [k, j1+j2-j3-k, j1-m1-k, j2+m2-k, j3-j2+m1+k, j3-j1-m2+k]
        if min(d) >= 0:
            s += (-1)**k / np.prod([float(f(t)) for t in d])
    return pref * s

def _u_c2r(l):
    U = np.zeros((2*l+1, 2*l+1), dtype=complex)
    U[l, l] = 1.0
    for m in range(1, l+1):
        U[l+m, l+m] = (-1)**m / sqrt(2); U[l+m, l-m] = 1.0/sqrt(2)
        U[l-m, l-m] = 1j/sqrt(2);        U[l-m, l+m] = -1j*(-1)**m/sqrt(2)
    return U

def _real_cg(l1, l2, l3):
    Cc = np.zeros((2*l1+1, 2*l2+1, 2*l3+1))
    for m1 in range(-l1, l1+1):
        for m2 in range(-l2, l2+1):
            for m3 in range(-l3, l3+1):
                Cc[m1+l1, m2+l2, m3+l3] = _su2_cg(l1, l2, l3, m1, m2, m3)
    T = np.einsum('ac,im,jn,mnc->aij', _u_c2r(l3), _u_c2r(l1).conj(), _u_c2r(l2).conj(), Cc)
    return (T.imag if np.abs(T.imag).max() > np.abs(T.real).max() else T.real)

def _real_sph_all(lmax, x, y, z):
    r = np.sqrt(x*x + y*y + z*z); eps = 1e-9
    ct = np.where(r > eps, z / np.maximum(r, eps), 1.0)
    phi = np.arctan2(y, x)
    st = np.sqrt(np.clip(1.0 - ct*ct, 0.0, None))
    P = {(0, 0): np.ones_like(ct)}
    for m in range(1, lmax+1):
        dfac = float(np.prod(np.arange(1, 2*m, 2)))
        P[(m, m)] = (-1)**m * dfac * st**m
    for m in range(0, lmax):
        P[(m+1, m)] = ct * (2*m+1) * P[(m, m)]
    for m in range(0, lmax+1):
        for l in range(m+2, lmax+1):
            P[(l, m)] = ((2*l-1)*ct*P[(l-1, m)] - (l+m-1)*P[(l-2, m)]) / (l - m)
    Y = {}
    for l in range(lmax+1):
        arr = np.zeros((2*l+1,) + ct.shape)
        for m in range(0, l+1):
            N = sqrt((2*l+1)/(4*pi) * factorial(l-m)/factorial(l+m))
            if m == 0:
                arr[l] = N * P[(l, 0)]
            else:
                arr[l+m] = sqrt(2)*N*P[(l, m)]*np.cos(m*phi)
                arr[l-m] = sqrt(2)*N*P[(l, m)]*np.sin(m*phi)
        if l > 0:
            arr = np.where(r[None] > eps, arr, 0.0)
        Y[l] = arr
    return Y

_g = np.arange(SIZE, dtype=np.float64) - PAD
_GX, _GY, _GZ = np.meshgrid(_g, _g, _g, indexing='ij')
_R = np.sqrt(_GX**2 + _GY**2 + _GZ**2)
_Y = _real_sph_all(4, _GX, _GY, _GZ)
_RAD = np.stack([np.exp(-0.5*(_R - c)**2) for c in np.linspace(0.0, PAD, N_RADIAL)]) * (_R <= PAD + 0.5)

def _build_basis(lo, li):
    ls = list(range(abs(lo-li), lo+li+1))
    out = []
    for l in ls:
        T = _real_cg(li, l, lo)
        ang = np.einsum('aim,mxyz->aixyz', T, _Y[l])
        for j in range(N_RADIAL):
            Bk = ang * _RAD[j]
            n = np.linalg.norm(Bk)
            out.append(Bk/n if n > 1e-12 else Bk)
    return np.stack(out).reshape(len(ls), N_RADIAL, 2*lo+1, 2*li+1, SIZE, SIZE, SIZE).astype(np.float32)

BASIS = {(lo, li): _build_basis(lo, li) for lo in range(3) for li in range(3)}
TPC = {l: _real_cg(1, 1, l).astype(np.float32) for l in range(3)}

# BN group matrix: 72 channels -> 24 multiplicity groups
_G = np.zeros((C_OUT, 3*M), dtype=np.float32)
for m_ in range(M):
    _G[m_, m_] = 1.0
    _G[M + 3*m_: M + 3*m_ + 3, M + m_] = 1.0
    _G[4*M + 5*m_: 4*M + 5*m_ + 5, 2*M + m_] = 1.0

# ---------------------------- host-side compute ----------------------------

def _build_K(ws):
    """ws: dict[(lo,li)] -> w [M, 2M, nl, 3]. Returns K [72, 144, 7,7,7]."""
    rows = []
    for lo in range(3):
        blocks = []
        for li in range(3):
            w = ws[(lo, li)]
            B = BASIS[(lo, li)]  # [nl, 3, 2lo+1, 2li+1, 7,7,7]
            Kb = np.einsum('uvlj,ljaixyz->uavixyz', w, B, optimize=True)
            blocks.append(Kb.reshape(M*(2*lo+1), 2*M*(2*li+1), SIZE, SIZE, SIZE))
        rows.append(np.concatenate(blocks, axis=1))
    return np.concatenate(rows, axis=0)

def _host_features(x):
    """x [2,72,20,20,20] -> padded xin [2,144,26,26,26] (float32)."""
    b = x.shape[0]
    s = x[:, :M]
    v = x[:, M:4*M].reshape(b, M, 3, D_, D_, D_)
    q = x[:, 4*M:].reshape(b, M, 5, D_, D_, D_)
    irr = [np.einsum('aij,buixyz,bujxyz->buaxyz', TPC[l], v, v, optimize=True)
           for l in range(3)]
    in0 = np.concatenate([s[:, :, None], irr[0]], 1).reshape(b, 2*M, D_, D_, D_)
    in1 = np.concatenate([v, irr[1]], 1).reshape(b, 2*M*3, D_, D_, D_)
    in2 = np.concatenate([q, irr[2]], 1).reshape(b, 2*M*5, D_, D_, D_)
    xin = np.concatenate([in0, in1, in2], 1)
    xp = np.zeros((b, C_MID, DP, DP, DP), dtype=np.float32)
    xp[:, :, PAD:PAD+D_, PAD:PAD+D_, PAD:PAD+D_] = xin
    return xp

# ------------------------------ device kernel ------------------------------

_BUILT = None

def _build_nc(reps=1):
    import concourse.bass as bass
    import concourse.bacc as bacc
    import concourse.mybir as mybir
    import concourse.tile as tile
    from contextlib import nullcontext

    f32 = mybir.dt.float32
    bf16 = mybir.dt.bfloat16
    NT2 = SIZE * SIZE                # 49
    NPLANE = 400                     # 20*20 output voxels per plane
    NQ = 8                           # K chunks: (18 chans x 7 dx) = 126 rows
    KC = 126
    CW = ZSLAB * DP * D_             # 5720 per-chunk free size

    nc = bacc.Bacc(None)
    xi_d = nc.declare_dram_parameter("xi", [NQ * KC, CW], bf16, isOutput=False)
    w_d = nc.declare_dram_parameter("w", [NQ * KC, NT2 * C_OUT], bf16, isOutput=False)
    gT_d = nc.declare_dram_parameter("gT", [C_OUT, 3*M], f32, isOutput=False)
    g2_d = nc.declare_dram_parameter("g2", [3*M, C_OUT], f32, isOutput=False)
    gam_d = nc.declare_dram_parameter("gamma", [3*M, 1], f32, isOutput=False)
    cst_d = nc.declare_dram_parameter("cst", [3*M, 2], f32, isOutput=False)
    bias_d = nc.declare_dram_parameter("bias", [M, 1], f32, isOutput=False)
    y_d = nc.declare_dram_parameter("y", [C_OUT, ZSH * NPLANE], f32, isOutput=True)

    with tile.TileContext(nc) as tc:
        with (
            tc.tile_pool(name="wpool", bufs=1) as wpool,
            tc.tile_pool(name="xpool", bufs=1) as xpool,
            tc.tile_pool(name="ypool", bufs=1) as ypool,
            tc.tile_pool(name="spool", bufs=1) as spool,
            tc.tile_pool(name="scratch", bufs=2) as scpool,
            tc.tile_pool(name="psum", bufs=1, space="PSUM") as ppool,
            tc.tile_pool(name="dram", bufs=1, space="DRAM") as dpool,
        ):
            # ---- persistent SBUF tensors ----
            w_sb = [wpool.tile([KC, NT2 * C_OUT], bf16, tag=f"w{q}", name=f"w{q}")
                    for q in range(NQ)]
            x_sb = [xpool.tile([KC, ZSLAB, DP, D_], bf16, tag=f"x{q}", name=f"x{q}")
                    for q in range(NQ)]
            y_sb = ypool.tile([C_OUT, ZSH * NPLANE], f32, tag="y")
            gT_sb = spool.tile([C_OUT, 3*M], f32, tag="gT")
            g2_sb = spool.tile([3*M, C_OUT], f32, tag="g2")
            gam_sb = spool.tile([3*M, 1], f32, tag="gam")
            cst_sb = spool.tile([3*M, 2], f32, tag="cst")
            bias_sb = spool.tile([M, 1], f32, tag="bias")
            ssq5 = spool.tile([C_OUT, ZSH], f32, tag="ssq5")
            ssq_t = spool.tile([C_OUT, 1], f32, tag="ssqt")
            ssq_ar = spool.tile([C_OUT, 1], f32, tag="ssqar")
            sc24 = spool.tile([3*M, 1], f32, tag="sc24")
            sc72 = spool.tile([C_OUT, 1], f32, tag="sc72")

            # ---- input DMAs (per K-chunk so compute starts early) ----
            for q in range(NQ):
                nc.sync.dma_start(w_sb[q][:], w_d[q*KC:(q+1)*KC, :])
                nc.sync.dma_start(x_sb[q][:].rearrange("p a b c -> p (a b c)"),
                                  xi_d[q*KC:(q+1)*KC, :])
            nc.sync.dma_start(gT_sb[:], gT_d[:])
            nc.sync.dma_start(g2_sb[:], g2_d[:])
            nc.sync.dma_start(gam_sb[:], gam_d[:])
            nc.sync.dma_start(cst_sb[:], cst_d[:])
            nc.sync.dma_start(bias_sb[:], bias_d[:])

            rep_ctx = tc.For_i(0, reps, 1) if reps > 1 else nullcontext()
            rep_ctx.__enter__()

            # ---- conv: accumulate into 5 PSUM banks (one per output plane) ----
            psum = [ppool.tile([C_OUT, NPLANE], f32, tag=f"ps{z}", name=f"ps{z}") for z in range(ZSH)]
            n_mm = NQ * NT2  # per-plane matmul count: 8 chunks x 49 (dz,dy)
            k = 0
            for q in range(NQ):
                for dz in range(SIZE):
                    for dy in range(SIZE):
                        t = dz*SIZE + dy
                        lhs = w_sb[q][:, t*C_OUT:(t+1)*C_OUT]
                        for zo in range(ZSH):
                            nc.tensor.matmul(
                                psum[zo][:], lhs,
                                x_sb[q][:, zo+dz, dy:dy+D_, :],
                                start=(k == 0), stop=(k == n_mm - 1))
                        k += 1

            # ---- evacuate + partial sum-of-squares ----
            for zo in range(ZSH):
                nc.vector.tensor_copy(y_sb[:, zo*NPLANE:(zo+1)*NPLANE], psum[zo][:])
                sq = scpool.tile([C_OUT, NPLANE], f32, tag="sq")
                nc.scalar.activation(sq[:], psum[zo][:],
                                     mybir.ActivationFunctionType.Square,
                                     accum_out=ssq5[:, zo:zo+1])
            nc.vector.reduce_sum(ssq_t[:], ssq5[:], axis=mybir.AxisListType.X)

            rep_ctx.__exit__(None, None, None)

            # ---- AllReduce the [72,1] partial stats ----
            ar_in = dpool.tile([C_OUT, 1], f32, tag="arin")
            ar_out = dpool.tile([C_OUT, 1], f32, tag="arout")
            nc.gpsimd.dma_start(ar_in[:], ssq_t[:])
            nc.gpsimd.collective_compute(
                "AllReduce", mybir.AluOpType.add,
                replica_groups=[list(range(N_CORES))],
                ins=[ar_in.opt()], outs=[ar_out.opt()])
            nc.gpsimd.dma_start(ssq_ar[:], ar_out[:])

            # ---- BN scale: group-sum, rsqrt, expand, apply ----
            g24 = ppool.tile([3*M, 1], f32, tag="g24")
            nc.tensor.matmul(g24[:], gT_sb[:], ssq_ar[:], start=True, stop=True)
            nc.vector.tensor_scalar(sc24[:], g24[:], cst_sb[:, 0:1], cst_sb[:, 1:2],
                                    op0=mybir.AluOpType.mult,
                                    op1=mybir.AluOpType.add)
            nc.scalar.activation(sc24[:], sc24[:],
                                 mybir.ActivationFunctionType.Sqrt)
            nc.vector.reciprocal(sc24[:], sc24[:])
            nc.vector.tensor_mul(sc24[:], sc24[:], gam_sb[:])
            s72 = ppool.tile([C_OUT, 1], f32, tag="s72")
            nc.tensor.matmul(s72[:], g2_sb[:], sc24[:], start=True, stop=True)
            nc.vector.tensor_copy(sc72[:], s72[:])
            nc.vector.tensor_scalar_mul(y_sb[:], y_sb[:], sc72[:, 0:1])
            nc.scalar.activation(y_sb[:M, :], y_sb[:M, :],
                                 mybir.ActivationFunctionType.Relu,
                                 bias=bias_sb[:, 0:1])

            nc.sync.dma_start(y_d[:], y_sb[:])

    if os.environ.get("KERNEL_LDW_DEDUP", "1") == "1":
        # Consecutive matmuls sharing the stationary operand skip the reload.
        nskip = 0
        for bb in nc.main_func.blocks:
            last = None
            for ins in bb.instructions:
                if type(ins).__name__ == "InstMatmult":
                    w = ins.ins[1]
                    key = (w.memref, w.offset)
                    if key == last:
                        ins.ldweights = False
                        nskip += 1
                    else:
                        last = key
        print(f"[kernel] ldweights dedup: skipped {nskip} reloads")

    nc.finalize()
    return nc

def _get_built(reps=1):
    global _BUILT
    if _BUILT is None:
        _BUILT = {}
    if reps not in _BUILT:
        _BUILT[reps] = _build_nc(reps)
    return _BUILT[reps]

# ------------------------------- entry point -------------------------------

def _prepare_in_maps(x, ws, bn_gamma, bias):
    x = np.asarray(x, dtype=np.float32)
    ws = {k: np.asarray(v, dtype=np.float32) for k, v in ws.items()}
    bn_gamma = np.asarray(bn_gamma, dtype=np.float32)
    bias = np.asarray(bias, dtype=np.float32)

    import ml_dtypes
    bf16 = ml_dtypes.bfloat16

    xp = _host_features(x)                       # [2,144,26,26,26]
    K = _build_K(ws).reshape(C_OUT, C_MID, SIZE, SIZE, SIZE)

    # lhsT layout: rows (c,dx) = c*7+dx, cols ((dz*7+dy)*72 + co)
    w = np.ascontiguousarray(
        K.transpose(1, 4, 2, 3, 0).reshape(C_MID * SIZE, SIZE*SIZE*C_OUT)
    ).astype(bf16)                                                # [1008, 49*72]

    # dx-replicated input (im2col over x): rows (c,dx), contiguous (y,x) blocks
    xr = np.ascontiguousarray(
        np.stack([xp[:, :, :, :, dx:dx+D_] for dx in range(SIZE)], axis=2)
        .reshape(B_, C_MID * SIZE, DP, DP, D_)).astype(bf16)      # [2,1008,26,26,20]

    gamma24 = bn_gamma.reshape(3*M, 1)
    bias8 = bias.reshape(M, 1)

    in_maps = []
    for k in range(N_CORES):
        b, zc = k // 4, k % 4
        z0 = ZSH * zc
        in_maps.append({
            "xi": np.ascontiguousarray(xr[b, :, z0:z0+ZSLAB]).reshape(C_MID*SIZE, -1),
            "w": w,
            "gT": _G, "g2": np.ascontiguousarray(_G.T),
            "gamma": gamma24, "bias": bias8,
            "cst": np.tile(np.array([[1.0/16000.0, 1e-5]], dtype=np.float32),
                           (3*M, 1)),
        })
    return in_maps


_LDW_PATCHED = False

def _maybe_patch_ldw_opt():
    """Optionally enable walrus's ldweights optimization (off by default in
    concourse) — overlaps/dedupes the per-matmul weight loads."""
    global _LDW_PATCHED
    import os
    if _LDW_PATCHED or os.environ.get("KERNEL_LDW_OPT", "0") != "1":
        return
    import concourse.bass_utils as _bu
    orig = _bu.run_command
    def patched(cmd, *a, **kw):
        if isinstance(cmd, list):
            cmd = ["--enable-ldw-opt=true" if c == "--enable-ldw-opt=false" else c
                   for c in cmd]
        return orig(cmd, *a, **kw)
    _bu.run_command = patched
    _LDW_PATCHED = True


def kernel(x, w00, w01, w02, w10, w11, w12, w20, w21, w22, bn_gamma, bias):
    from concourse.bass_utils import run_bass_kernel_spmd
    _maybe_patch_ldw_opt()

    ws = {(0,0): w00, (0,1): w01, (0,2): w02, (1,0): w10, (1,1): w11,
          (1,2): w12, (2,0): w20, (2,1): w21, (2,2): w22}
    in_maps = _prepare_in_maps(x, ws, bn_gamma, bias)
    nc = _get_built()
    res = run_bass_kernel_spmd(nc, in_maps, list(range(N_CORES)))

    out = np.empty((B_, C_OUT, D_, D_, D_), dtype=np.float32)
    for k in range(N_CORES):
        b, zc = k // 4, k % 4
        z0 = ZSH * zc
        out[b, :, z0:z0+ZSH] = res.results[k]["y"].reshape(C_OUT, ZSH, D_, D_)
    return out
